# revision 1
# baseline (speedup 1.0000x reference)
"""DualAttentionBlock (DANet) Bass kernel for 8 TRN2 NeuronCores.

Problem: x [4, 512, 64, 64] f32; position attention (NxN over 4096 positions,
Cq=32 query/key channels) + channel attention (CxC over 512 channels), summed
with two residuals:  out = (gpa*PA(x) + x) + (gca*CA(x) + x).

Sharding: 8 cores = 4 batches x 2 query-halves. Each core receives the full
[512, 4096] slab of its batch with the spatial axis rotated so its 2048 query
positions come first (both attentions are permutation-invariant along the
contracted/key axis), computes attention for those 2048 positions, and writes
out [512, 2048]. The host reassembles the halves.

Precision: the big matmuls run as float32r (TF32-like, ~1.5e-4) or bf16
(energy in PSUM fp32 always); softmax/normalization in fp32; the dominant
2*x residual term is exact fp32.
"""
import os
os.environ.setdefault("JAX_PLATFORMS", "")
import numpy as np

import concourse.bass as bass
import concourse.mybir as mybir
import concourse.tile as tile
from concourse import bacc
from concourse.bass_utils import run_bass_kernel_spmd

P = 128
C = 512
CQ = 32
NF = 4096   # full spatial positions (keys)
NQ = 2048   # query positions per core
CCH = C // P      # 4 channel chunks
MCH = NF // P     # 32 key chunks
NS = NQ // 512    # 4 query super-blocks

F32 = mybir.dt.float32
F32R = mybir.dt.float32r
BF16 = mybir.dt.bfloat16
AF = mybir.ActivationFunctionType
OP = mybir.AluOpType


def _body(nc, tc, ctx, reps=1):
    xf = nc.dram_tensor("xf", [C, NF], F32, kind="ExternalInput").ap()
    wqT = nc.dram_tensor("wqT", [C, CQ], F32, kind="ExternalInput").ap()
    wkT = nc.dram_tensor("wkT", [C, CQ], F32, kind="ExternalInput").ap()
    wvT = nc.dram_tensor("wvT", [C, C], F32, kind="ExternalInput").ap()
    bq = nc.dram_tensor("bq", [CQ, 1], F32, kind="ExternalInput").ap()
    bk = nc.dram_tensor("bk", [P, 1], F32, kind="ExternalInput").ap()
    bv = nc.dram_tensor("bv", [1, C], F32, kind="ExternalInput").ap()
    gpa = nc.dram_tensor("gpa", [P, 1], F32, kind="ExternalInput").ap()
    gca = nc.dram_tensor("gca", [P, 1], F32, kind="ExternalInput").ap()
    onec = nc.dram_tensor("onec", [P, 1], F32, kind="ExternalInput").ap()
    oner = nc.dram_tensor("oner", [1, P], F32, kind="ExternalInput").ap()
    ident = nc.dram_tensor("ident", [P, P], F32, kind="ExternalInput").ap()
    out = nc.dram_tensor("out", [C, NQ], F32, kind="ExternalOutput").ap()

    sb = ctx.enter_context(tc.tile_pool(name="sb", bufs=1))
    ps = ctx.enter_context(tc.tile_pool(name="ps", bufs=1, space="PSUM"))
    if reps > 1:
        ctx.enter_context(tc.For_i(0, reps, 1))

    # ---------------- loads ----------------
    xf_lo = sb.tile([P, CCH, NQ], F32, tag="BIGA")
    xf_hi = sb.tile([P, CCH, NQ], F32, tag="BIGB")
    wq_sb = sb.tile([P, CCH, CQ], F32, tag="wq")
    wk_sb = sb.tile([P, CCH, CQ], F32, tag="wk")
    id_sb = sb.tile([P, P], F32, tag="ident")
    wv_sb = sb.tile([P, CCH, C], F32, tag="wv")
    bq_sb = sb.tile([CQ, 1], F32, tag="bq")
    bk_sb = sb.tile([P, 1], F32, tag="bk")
    bv_sb = sb.tile([1, C], F32, tag="bv")
    gpa_sb = sb.tile([P, 1], F32, tag="gpa")
    gca_sb = sb.tile([P, 1], F32, tag="gca")
    onec_sb = sb.tile([P, 1], F32, tag="onec")
    oner_sb = sb.tile([1, P], F32, tag="oner")
    nc.sync.dma_start(id_sb[:], ident)
    nc.sync.dma_start(wq_sb[:], wqT.rearrange("(j p) o -> p j o", p=P))
    nc.sync.dma_start(wk_sb[:], wkT.rearrange("(j p) o -> p j o", p=P))
    xf3 = xf.rearrange("(j p) n -> p j n", p=P)
    for g4 in range(4):
        nc.sync.dma_start(xf_lo[:, :, g4 * 512:(g4 + 1) * 512],
                          xf3[:, :, g4 * 512:(g4 + 1) * 512])
    # small operands needed mid-kernel: slot their transfers between the
    # xf halves so they don't queue behind 17 MB on the DMA engines
    nc.sync.dma_start(onec_sb[:], onec)
    nc.sync.dma_start(oner_sb[:], oner)
    nc.sync.dma_start(bv_sb[:], bv)
    nc.sync.dma_start(wv_sb[:], wvT.rearrange("(j p) o -> p j o", p=P))
    for g4 in range(4):
        nc.sync.dma_start(xf_hi[:, :, g4 * 512:(g4 + 1) * 512],
                          xf3[:, :, NQ + g4 * 512:NQ + (g4 + 1) * 512])

    nc.gpsimd.dma_start(bq_sb[:], bq)
    nc.gpsimd.dma_start(bk_sb[:], bk)
    nc.gpsimd.dma_start(gpa_sb[:], gpa)
    nc.gpsimd.dma_start(gca_sb[:], gca)

    # ---------------- rounded copies of small operands ----------------
    id_r = sb.tile([P, P], F32R, tag="idr")
    id_b = sb.tile([P, P], BF16, tag="idb")
    nc.vector.tensor_copy(id_r[:], id_sb[:])
    nc.vector.tensor_copy(id_b[:], id_sb[:])

    # PE warmup during the input-DMA wait: keeps the HAM clock-gate hot so
    # the first real matmuls run at full rate. Results are consumed by one
    # ACT copy into a scratch tile nothing reads.
    pwarm = ps.tile([P, P], F32, tag="psm", bufs=1)
    for _ in range(16):
        nc.tensor.matmul(pwarm[:], id_r[:], id_r[:], start=True, stop=True)
    warm_sink = sb.tile([P, P], BF16, tag="warmsink")
    nc.scalar.activation(warm_sink[:], pwarm[:], AF.Copy)
    wq_r = sb.tile([P, CCH, CQ], F32R, tag="wqr")
    wk_r = sb.tile([P, CCH, CQ], F32R, tag="wkr")
    nc.vector.tensor_copy(wq_r[:], wq_sb[:])
    nc.vector.tensor_copy(wk_r[:], wk_sb[:])

    # ---------------- early: q, k, xfT (transposes) ----------------
    q_sb = sb.tile([CQ, NQ], BF16, tag="q")
    k_sb = sb.tile([CQ, NF], BF16, tag="k")
    xfT = sb.tile([P, MCH, C], F32R, tag="BIGT")

    def g_block(g):
        src = xf_lo if g < 4 else xf_hi
        c0 = (g % 4) * 512
        rt = sb.tile([P, CCH, 512], F32R, tag="rt", bufs=2, name=f"rt_{g}")
        nc.vector.tensor_copy(rt[:], src[:, :, c0:c0 + 512])
        if g < 4:
            pq = ps.tile([CQ, 512], F32, tag="pst", bufs=3, name=f"pq_{g}")
            for c in range(CCH):
                nc.tensor.matmul(pq[:], wq_r[:, c, :], rt[:, c, :],
                                 start=(c == 0), stop=(c == CCH - 1))
            nc.vector.tensor_scalar_add(q_sb[:, g * 512:(g + 1) * 512], pq[:], bq_sb[:])
        pk = ps.tile([CQ, 512], F32, tag="pst", bufs=3, name=f"pk_{g}")
        for c in range(CCH):
            nc.tensor.matmul(pk[:], wk_r[:, c, :], rt[:, c, :],
                             start=(c == 0), stop=(c == CCH - 1))
        nc.vector.tensor_scalar_add(k_sb[:, g * 512:(g + 1) * 512], pk[:], bk_sb[0:CQ, :])
        for nb in range(4):
            ptp = ps.tile([P, 512], F32R, tag="pst", bufs=3, name=f"ptp_{g}_{nb}")
            for c in range(CCH):
                nc.tensor.transpose(ptp[:, c * P:(c + 1) * P],
                                    rt[:, c, nb * P:(nb + 1) * P], id_r[:])
            nc.scalar.activation(xfT[:, g * 4 + nb, :], ptp[:], AF.Copy)

    attn_n = sb.tile([P, CCH, C], BF16, tag="attn_n")
    pe_blocks = []
    with nc.named_scope("early_qk_xfT_lo"):
        for g in range(4):
            g_block(g)
    # CA-energy over the lo half while the hi half still loads
    with nc.named_scope("ca_energy_lo"):
        for cb in range(CCH):
            pe_ = ps.tile([P, C], F32, tag="pacc", bufs=4, name=f"pe_{cb}")
            pe_blocks.append(pe_)
            for nch in range(16):
                nc.tensor.matmul(pe_[:], xfT[:, nch, cb * P:(cb + 1) * P], xfT[:, nch, :],
                                 start=(nch == 0), stop=False)
    with nc.named_scope("early_qk_xfT_hi"):
        for g in range(4, 8):
            g_block(g)

    # ---------------- channel attention: energy + softmax ----------------
    with nc.named_scope("ca_energy_softmax"):
        for cb in range(CCH):
            pe_ = pe_blocks[cb]
            for nch in range(16, MCH):
                nc.tensor.matmul(pe_[:], xfT[:, nch, cb * P:(cb + 1) * P], xfT[:, nch, :],
                                 start=False, stop=(nch == MCH - 1))
            rowmin = sb.tile([P, 1], F32, tag="rowmin", bufs=1)
            nc.vector.tensor_reduce(rowmin[:], pe_[:], axis=mybir.AxisListType.X, op=OP.min)
            attn_un = sb.tile([P, C], BF16, tag="attn_un", bufs=1)
            rowsum = sb.tile([P, 1], F32, tag="rowsum", bufs=1)
            # attn = exp(rowmin - e); softmax(max-e) == softmax(-e) and
            # exp(rowmin - e) is the max-stabilized form of exp(-e).
            nc.scalar.activation(attn_un[:], pe_[:], AF.Exp, scale=-1.0,
                                 bias=rowmin[:], accum_out=rowsum[:])
            recip = sb.tile([P, 1], F32, tag="recip", bufs=1)
            nc.vector.reciprocal(recip[:], rowsum[:])
            nc.vector.tensor_mul(recip[:], recip[:], gca_sb[:])
            nc.scalar.activation(attn_n[:, cb, :], attn_un[:], AF.Copy, scale=recip[:])

    # ---------------- late small operands (wv, bv, ones, neg8) ----------------
    wv_b = sb.tile([P, CCH, C], BF16, tag="wvb")
    nc.vector.tensor_copy(wv_b[:], wv_sb[:])
    bv_r = sb.tile([1, C], F32R, tag="rowvec")
    nc.vector.tensor_copy(bv_r[:], bv_sb[:])
    onec_b = sb.tile([P, 1], BF16, tag="onecb")
    oner_r = sb.tile([1, P], F32R, tag="onerr")
    nc.vector.tensor_copy(onec_b[:], onec_sb[:])
    nc.vector.tensor_copy(oner_r[:], oner_sb[:])
    neg8 = sb.tile([P, 1], F32, tag="neg8")
    nc.gpsimd.memset(neg8[:], -8.0)
    # bv broadcast to all partitions: [128, C] = ones[1,128].T @ bv[1,C]
    pbv = ps.tile([P, C], F32, tag="pst", bufs=3)
    nc.tensor.matmul(pbv[:], oner_r[:], bv_r[:], start=True, stop=True)
    bv_bc = sb.tile([P, C], BF16, tag="bvbc")
    nc.scalar.activation(bv_bc[:], pbv[:], AF.Copy)

    # ---------------- vT = (wv @ xf + bv)^T  [NF, C] ----------------
    vT = sb.tile([P, MCH, C], F32R, tag="BIGT")
    with nc.named_scope("v_transposed"):
        for g in range(8):
            src = xf_lo if g < 4 else xf_hi
            c0 = (g % 4) * 512
            rt = sb.tile([P, CCH, 512], BF16, tag="rt", bufs=2)
            nc.vector.tensor_copy(rt[:], src[:, :, c0:c0 + 512])
            for nb in range(4):
                pv = ps.tile([P, C], F32, tag="pacc", bufs=4)
                for c in range(CCH):
                    nc.tensor.matmul(pv[:], rt[:, c, nb * P:(nb + 1) * P], wv_b[:, c, :],
                                     start=(c == 0), stop=(c == CCH - 1))
                nc.vector.tensor_add(vT[:, g * 4 + nb, :], pv[:], bv_bc[:])

    # ---------------- attn^T for CA out ----------------
    attnT = sb.tile([P, CCH, C], BF16, tag="attnT")
    with nc.named_scope("ca_attnT"):
        for d in range(CCH):
            ptb = ps.tile([P, C], BF16, tag="pst", bufs=3)
            for cb in range(CCH):
                nc.tensor.transpose(ptb[:, cb * P:(cb + 1) * P],
                                    attn_n[:, cb, d * P:(d + 1) * P], id_b[:])
            nc.scalar.activation(attnT[:, d, :], ptb[:], AF.Copy)

    # ---------------- CA out + residuals: ca_term = gca*CA + 2x ----------------
    ca_term = sb.tile([P, CCH, NQ], F32, tag="BIGB")
    with nc.named_scope("ca_out"):
        for nb4 in range(NS):
            rtb = sb.tile([P, CCH, 512], BF16, tag="rtb", bufs=1)
            nc.vector.tensor_copy(rtb[:], xf_lo[:, :, nb4 * 512:(nb4 + 1) * 512])
            for cb in range(CCH):
                pco = ps.tile([P, 512], F32, tag="pacc", bufs=4)
                for d in range(CCH):
                    nc.tensor.matmul(pco[:], attnT[:, d, cb * P:(cb + 1) * P], rtb[:, d, :],
                                     start=(d == 0), stop=(d == CCH - 1))
                nc.vector.scalar_tensor_tensor(
                    ca_term[:, cb, nb4 * 512:(nb4 + 1) * 512],
                    xf_lo[:, cb, nb4 * 512:(nb4 + 1) * 512], 2.0, pco[:],
                    op0=OP.mult, op1=OP.add)

    # ---------------- position attention ----------------
    with nc.named_scope("pa"):
        for s in range(NS):
            pouts = []
            for cb in range(CCH):
                po = ps.tile([P, 512], F32, tag="pacc", bufs=4, name=f"po_{s}_{cb}")
                pouts.append(po)
            prs = ps.tile([1, 512], F32, tag="psm", bufs=1)
            ex_prev = None
            for m in range(MCH):
                pst_ = ps.tile([P, 512], F32, tag="pst", bufs=3)
                nc.tensor.matmul(pst_[:], k_sb[:, m * P:(m + 1) * P],
                                 q_sb[:, s * 512:(s + 1) * 512], start=True, stop=True)
                ex = sb.tile([P, 512], F32R, tag="ex", bufs=2, name=f"ex_{s}_{m}")
                # exp(S - 8): constant shift cancels in normalization,
                # guards overflow without a max pass.
                nc.scalar.activation(ex[:], pst_[:], AF.Exp, scale=1.0, bias=neg8[:])
                for cb in range(CCH):
                    nc.tensor.matmul(pouts[cb][:], vT[:, m, cb * P:(cb + 1) * P], ex[:],
                                     start=(m == 0), stop=(m == MCH - 1))
                if m % 2 == 0:
                    ex_prev = ex
                else:
                    # pair-sum on DVE; quad-sum in bf16 (2x DVE mode); one
                    # rowsum matmul per quad (PE saving)
                    exs = sb.tile([P, 512], BF16, tag="exs", bufs=2, name=f"exs_{s}_{m}")
                    nc.vector.tensor_add(exs[:], ex_prev[:], ex[:])
                    if m % 4 == 1:
                        exs_prev = exs
                    else:
                        exq = sb.tile([P, 512], BF16, tag="exq", bufs=1, name=f"exq_{s}_{m}")
                        nc.vector.tensor_add(exq[:], exs_prev[:], exs[:])
                        nc.tensor.matmul(prs[:], onec_b[:], exq[:],
                                         start=(m == 3), stop=(m == MCH - 1))
            rr_r = sb.tile([1, 512], F32R, tag="rowvec", bufs=1)
            with nc.allow_low_precision(reason="f32r softmax denom, ~1e-4"):
                nc.vector.reciprocal(rr_r[:], prs[:])
            pbc = ps.tile([P, 512], F32, tag="pst", bufs=3)
            nc.tensor.matmul(pbc[:], oner_r[:], rr_r[:], start=True, stop=True)
            bc = sb.tile([P, 512], F32, tag="bc", bufs=1)
            nc.scalar.activation(bc[:], pbc[:], AF.Copy)
            for cb in range(CCH):
                fin = sb.tile([P, 512], F32, tag="fin", bufs=3)
                nc.vector.tensor_mul(fin[:], pouts[cb][:], bc[:])
                # fin = fin * gpa + ca_term  (gamma fold + residual in one op)
                nc.vector.scalar_tensor_tensor(
                    fin[:], fin[:], gpa_sb[:], ca_term[:, cb, s * 512:(s + 1) * 512],
                    op0=OP.mult, op1=OP.add)
                nc.sync.dma_start(out[cb * P:(cb + 1) * P, s * 512:(s + 1) * 512], fin[:])


_NC_CACHE = {}


def _get_nc(reps=1):
    if reps not in _NC_CACHE:
        nc = bacc.Bacc("TRN2", target_bir_lowering=False, debug=False)
        from contextlib import ExitStack
        with tile.TileContext(nc) as tc, ExitStack() as ctx:
            _body(nc, tc, ctx, reps=reps)
        nc.compile()
        _NC_CACHE[reps] = nc
    return _NC_CACHE[reps]


def _in_maps(x, wq, bq, wk, bk, wv, bv, gamma_pa, gamma_ca):
    B = x.shape[0]
    xb = np.ascontiguousarray(x.reshape(B, C, NF)).astype(np.float32)
    base = {
        "wqT": np.ascontiguousarray(wq.T).astype(np.float32),
        "wkT": np.ascontiguousarray(wk.T).astype(np.float32),
        "wvT": np.ascontiguousarray(wv.T).astype(np.float32),
        "bq": np.asarray(bq, np.float32).reshape(CQ, 1),
        "bk": np.tile(np.asarray(bk, np.float32).reshape(CQ, 1), (4, 1)),
        "bv": np.asarray(bv, np.float32).reshape(1, C),
        "gpa": np.full((P, 1), np.float32(np.asarray(gamma_pa).reshape(-1)[0])),
        "gca": np.full((P, 1), np.float32(np.asarray(gamma_ca).reshape(-1)[0])),
        "onec": np.ones((P, 1), np.float32),
        "oner": np.ones((1, P), np.float32),
        "ident": np.eye(P, dtype=np.float32),
    }
    maps = []
    for i in range(8):
        b, h = i // 2, i % 2
        sl = xb[b]
        if h == 0:
            xperm = sl
        else:
            xperm = np.concatenate([sl[:, NQ:], sl[:, :NQ]], axis=1)
        m = dict(base)
        m["xf"] = np.ascontiguousarray(xperm)
        maps.append(m)
    return maps


def _run(inputs, trace=False):
    nc = _get_nc()
    maps = _in_maps(**inputs)
    res = run_bass_kernel_spmd(nc, maps, core_ids=list(range(8)), trace=trace)
    B = inputs["x"].shape[0]
    full = np.empty((B, C, NF), np.float32)
    for i in range(8):
        b, h = i // 2, i % 2
        full[b][:, h * NQ:(h + 1) * NQ] = res.results[i]["out"]
    return full.reshape(inputs["x"].shape), res


def kernel(**inputs) -> np.ndarray:
    out, _ = _run(inputs, trace=False)
    return out


def _make_fn(nc, maps):
    import jax
    from jax.sharding import Mesh, PartitionSpec
    from concourse import bass2jax
    from concourse.bass2jax import shard_map

    bass2jax.install_neuronx_cc_hook()
    n_cores = 8
    pname = nc.partition_id_tensor.name if nc.partition_id_tensor else None
    in_names, out_names, out_avals, zero_outs = [], [], [], []
    for alloc in nc.m.functions[0].allocations:
        if not isinstance(alloc, mybir.MemoryLocationSet):
            continue
        name = alloc.memorylocations[0].name
        if alloc.kind == "ExternalInput":
            if name != pname:
                in_names.append(name)
        elif alloc.kind == "ExternalOutput":
            shape = tuple(alloc.tensor_shape)
            dtype = mybir.dt.np(alloc.dtype)
            out_names.append(name)
            out_avals.append(jax.core.ShapedArray(shape, dtype))
            zero_outs.append(np.zeros(shape, dtype))
    n_params = len(in_names)
    all_names = in_names + out_names + ([pname] if pname else [])

    def _body(*args):
        operands = list(args)
        if pname:
            operands.append(bass2jax.partition_id_tensor())
        outs = bass2jax._bass_exec_p.bind(
            *operands,
            out_avals=tuple(out_avals),
            in_names=tuple(all_names),
            out_names=tuple(out_names),
            lowering_input_output_aliases=(),
            sim_require_finite=True,
            sim_require_nnan=True,
            nc=nc,
        )
        return tuple(outs)

    devices = jax.devices()[:n_cores]
    mesh = Mesh(np.asarray(devices), ("core",))
    n_outs = len(out_names)
    fn = jax.jit(
        shard_map(_body, mesh=mesh,
                  in_specs=(PartitionSpec("core"),) * (n_params + n_outs),
                  out_specs=(PartitionSpec("core"),) * n_outs,
                  check_rep=False),
        keep_unused=True,
    )
    concat_in = [np.concatenate([np.asarray(maps[c][nm]) for c in range(n_cores)], axis=0)
                 for nm in in_names]
    concat_zero = [np.zeros((n_cores * z.shape[0], *z.shape[1:]), z.dtype)
                   for z in zero_outs]
    sharding = jax.sharding.NamedSharding(mesh, PartitionSpec("core"))
    dev_in = [jax.device_put(a, sharding) for a in concat_in]
    dev_zero = [jax.device_put(a, sharding) for a in concat_zero]
    return lambda: fn(*dev_in, *dev_zero)


def bench_fn(inputs, reps=1):
    nc = _get_nc(reps)
    maps = _in_maps(**inputs)
    return _make_fn(nc, maps)


def assemble(inputs, outs):
    res = np.asarray(outs[0]).reshape(8, C, NQ)
    B = inputs["x"].shape[0]
    full = np.empty((B, C, NF), np.float32)
    for i in range(8):
        b, h = i // 2, i % 2
        full[b][:, h * NQ:(h + 1) * NQ] = res[i]
    return full.reshape(inputs["x"].shape)


def bench(inputs, iters=10, reps=1):
    import time as _time
    import jax
    fn = bench_fn(inputs, reps=reps)
    outs = fn()
    jax.block_until_ready(outs)
    times = []
    for _ in range(iters):
        t0 = _time.perf_counter()
        outs = fn()
        jax.block_until_ready(outs)
        times.append(_time.perf_counter() - t0)
    best = min(times)
    mean = sum(times) / len(times)
    return int(best * 1e9), int(mean * 1e9), assemble(inputs, outs)

